# revision 1
# baseline (speedup 1.0000x reference)
"""Trainium2 Bass kernel for nn_Efficient8BitALU_AddSub.

Contract: kernel(**inputs) takes FULL unsharded inputs (numpy), returns FULL
output [32, 2048, 128] float32.  Internally shards tokens across 8 NeuronCores
(pure data parallel), runs a Bass/Tile kernel per core, gathers.

Per-core pipeline (TPC tokens = NT tiles of 128 tokens):
  DVE   decode: tsel=(x>0.5)*(k-16); min-reduce per 16-window; sentinel fix
  DVE   flags/masks; assemble per-token component vector c (bf16):
          [a, b, opA, opS, 1, a*mA, b*mA, opA*mA, opS*mA, mA] duplicated
          twice (rows 0-9 pair with Whi, rows 10-19 with Wlo = W - bf16(W))
  PE    transpose c -> comp-partitioned;  h = W16^T c  (K=20 bf16, exact to
          ~1e-5 via the hi/lo weight split;  W16 encodes
          Wsub-eff ; (Wadd-Wsub)-eff so h = h_selected directly)
  ACT   relu psum->SBUF as fp16
  PE    layer2: res[tok,2] = relu_h^T @ [w2_add, w2_sub]  (fp16, exact in
          fp32 psum given fp16 inputs)
  DVE   select by is_add, round (RNE +C-C trick), clamp, fold processed mask
          into the index, one-hot is_equal vs iota, fused scatter-add into x
  DMA   contiguous 128-token-tile loads/stores, 16 chunks each way
"""

import sys

import numpy as np

sys.path.insert(0, "/opt/trn_rl_repo")

import ml_dtypes  # noqa: E402
import concourse.bacc as bacc  # noqa: E402
import concourse.bass as bass  # noqa: E402
import concourse.mybir as mybir  # noqa: E402
import concourse.tile as tile  # noqa: E402

dt = mybir.dt
Alu = mybir.AluOpType
Act = mybir.ActivationFunctionType

# ---- problem constants (hardcoded per contract) ----
B, S, D = 32, 2048, 128
NCORES = 8
TOK = B * S                   # 65536
TPC = TOK // NCORES           # 8192 tokens per core

MARK_AX, OP_ADD, OP_SUB = 0, 1, 2
WIN0 = 3                      # 4 contiguous 16-wide decode windows: 3..66
OUT_LO = 67                   # outputs 67..98 (lo 67:83, hi 83:99)
OPA, OPS = 124, 125
GE_RESULT = 63
ROUND_C = 12582912.0          # 1.5 * 2**23 : RNE round-to-integer magic

G = 8                         # tiles per supertile (multiple of 8)


def build_nc(tpc=TPC, g=G):
    nt = tpc // 128
    nst = nt // g
    assert nt % g == 0 and g % 8 == 0

    nc = bacc.Bacc("TRN2", target_bir_lowering=False, debug=False,
                   num_devices=NCORES)
    xd = nc.dram_tensor("xc", [tpc, D], dt.float32, kind="ExternalInput")
    w16d = nc.dram_tensor("cW16", [128, 128], dt.bfloat16, kind="ExternalInput")
    w2d = nc.dram_tensor("cW2", [128, 2], dt.float16, kind="ExternalInput")
    iotad = nc.dram_tensor("cIOTA", [128, 32], dt.float32, kind="ExternalInput")
    k16d = nc.dram_tensor("cK16", [128, 64], dt.float32, kind="ExternalInput")
    idd = nc.dram_tensor("cID", [128, 128], dt.bfloat16, kind="ExternalInput")
    yd = nc.dram_tensor("yc", [tpc, D], dt.float32, kind="ExternalOutput")

    xr = xd.ap().rearrange("(n p) f -> p n f", p=128)
    yr = yd.ap().rearrange("(n p) f -> p n f", p=128)

    ndma = nt // g                # one DMA chunk per supertile (FIFO-ordered)
    tiles_per_dma = g

    with tile.TileContext(nc) as tc:
        with (
            tc.tile_pool(name="const", bufs=1) as cpool,
            tc.tile_pool(name="xbuf", bufs=1) as xpool,
            tc.tile_pool(name="work", bufs=3) as wpool,
            tc.tile_pool(name="ctp", bufs=2, space="PSUM") as ctp_pool,
            tc.tile_pool(name="hp", bufs=2, space="PSUM") as hp_pool,
            tc.tile_pool(name="rp", bufs=2, space="PSUM") as rp_pool,
        ):
            W16 = cpool.tile([128, 128], dt.bfloat16, tag="w16")
            W2 = cpool.tile([128, 2], dt.float16, tag="w2")
            IOTA = cpool.tile([128, 32], dt.float32, tag="iota")
            IDN = cpool.tile([128, 128], dt.bfloat16, tag="idn")
            # constants via SWDGE (gpsimd) — keeps the HWDGE ring free for
            # the big input chunks
            nc.gpsimd.dma_start(W16[:], w16d.ap())
            nc.gpsimd.dma_start(W2[:], w2d.ap())
            nc.gpsimd.dma_start(IOTA[:], iotad.ap())
            nc.gpsimd.dma_start(IDN[:], idd.ap())

            X = xpool.tile([128, nt * 128], dt.float32, tag="X")
            XR = X[:].rearrange("p (n f) -> p n f", f=128)

            K16S = xpool.tile([128, g * 64], dt.float32, tag="K16S")
            nc.gpsimd.dma_start(
                K16S[:].rearrange("p (n k) -> p n k", k=64),
                k16d.ap()[:, None, :].to_broadcast([128, g, 64]))

            # c staging (bf16), manual double-buffer so pads/ones survive
            cbs = []
            for i in range(3):
                cb = xpool.tile([128, g * 64], dt.bfloat16, tag=f"CB{i}",
                                name=f"CB{i}")
                nc.vector.memset(cb[:], 0.0)
                cb4 = cb[:].rearrange("p (n s c) -> p n s c", s=2, c=32)
                nc.vector.memset(cb4[:, :, :, 4:5], 1.0)
                cbs.append(cb)

            cts = [xpool.tile([64, g * 128], dt.bfloat16, tag=f"CT{i}",
                              name=f"CT{i}") for i in range(3)]
            rhs_ = [xpool.tile([128, g * 256], dt.float16, tag=f"RH{i}",
                               name=f"RH{i}") for i in range(3)]

            # input: 2 half-chunks per supertile in 2 parallel FIFO chains so
            # supertile k's data lands before supertile k+1's
            half = tiles_per_dma // 2
            prev_in = [None, None]
            for d_ in range(ndma):
                for piece in range(2):
                    t0 = d_ * tiles_per_dma + piece * half
                    di = nc.sync.dma_start(XR[:, t0:t0 + half, :],
                                           xr[:, t0:t0 + half, :])
                    if prev_in[piece] is not None:
                        tile.add_dep_helper(di.ins, prev_in[piece].ins,
                                            reason="input chunk ordering")
                    prev_in[piece] = di

            for st in range(nst):
                T0 = st * g
                CB = cbs[st % 3]
                CB4 = CB[:].rearrange("p (n s c) -> p n s c", s=2, c=32)
                CT = cts[st % 3]
                RH = rhs_[st % 3]

                # ---------- decode ----------
                TSEL = wpool.tile([128, g * 64], dt.float32, tag="tsel")
                nc.vector.scalar_tensor_tensor(
                    out=TSEL[:],
                    in0=XR[:, T0:T0 + g, WIN0:WIN0 + 64],
                    scalar=0.5,
                    in1=K16S[:].rearrange("p (n k) -> p n k", k=64),
                    op0=Alu.is_gt, op1=Alu.mult)
                NIB = wpool.tile([128, g * 4], dt.float32, tag="nib")
                nc.vector.tensor_reduce(
                    out=NIB[:],
                    in_=TSEL[:].rearrange("p (n w k) -> p n w k", w=4, k=16),
                    axis=mybir.AxisListType.X, op=Alu.min)
                # (min+16) with no-hit sentinel (0 -> 16) mapped back to 0
                NM = wpool.tile([128, g * 4], dt.float32, tag="nm")
                nc.vector.tensor_scalar(out=NM[:], in0=NIB[:], scalar1=-0.5,
                                        scalar2=None, op0=Alu.is_lt)
                nc.vector.scalar_tensor_tensor(out=NIB[:], in0=NIB[:],
                                               scalar=16.0, in1=NM[:],
                                               op0=Alu.add, op1=Alu.mult)
                NIB4 = NIB[:].rearrange("p (n w) -> p n w", w=4)
                NIBV = NIB4.rearrange("p n (ab pos) -> p n pos ab", pos=2)

                # ---------- flags ----------
                MA = wpool.tile([128, g], dt.float32, tag="ma")
                ISS = wpool.tile([128, g], dt.float32, tag="iss")
                A2 = wpool.tile([128, g], dt.float32, tag="a2")
                M2 = wpool.tile([128, g], dt.float32, tag="m2")
                nc.vector.tensor_scalar(out=MA[:], in0=XR[:, T0:T0 + g, OP_ADD],
                                        scalar1=0.5, scalar2=None, op0=Alu.is_gt)
                nc.vector.tensor_scalar(out=ISS[:], in0=XR[:, T0:T0 + g, OP_SUB],
                                        scalar1=0.5, scalar2=None, op0=Alu.is_gt)
                nc.vector.tensor_scalar(out=A2[:], in0=XR[:, T0:T0 + g, MARK_AX],
                                        scalar1=0.5, scalar2=None, op0=Alu.is_gt)
                nc.vector.tensor_tensor(out=M2[:], in0=MA[:], in1=ISS[:],
                                        op=Alu.max)
                nc.vector.scalar_tensor_tensor(out=M2[:], in0=M2[:], scalar=2.0,
                                               in1=A2[:], op0=Alu.mult,
                                               op1=Alu.mult)

                MAb22 = MA[:, :, None, None].broadcast_to([128, g, 2, 2])
                OPV = XR[:, T0:T0 + g, OPA:OPS + 1][:, :, None, :] \
                    .broadcast_to([128, g, 2, 2])

                # ---------- c build (bf16) ----------
                # rows 0..9:  [a,b,opA,opS,1, a*mA,b*mA,opA*mA,opS*mA, mA]
                # rows 10..13: dup of [a,b, a*mA,b*mA] (pair with Wlo rows;
                #   op/bias/mask Wlo terms are negligible and skipped)
                nc.vector.tensor_copy(CB4[:, :, :, 0:2], NIBV)
                nc.vector.tensor_copy(CB4[:, :, :, 2:4], OPV)
                nc.vector.tensor_tensor(out=CB4[:, :, :, 5:7],
                                        in0=NIBV, in1=MAb22, op=Alu.mult)
                nc.vector.tensor_tensor(out=CB4[:, :, :, 7:9],
                                        in0=OPV, in1=MAb22, op=Alu.mult)
                nc.vector.tensor_copy(
                    CB4[:, :, :, 9:10],
                    MA[:, :, None, None].broadcast_to([128, g, 2, 1]))
                nc.vector.tensor_copy(CB4[:, :, :, 10:12], NIBV)
                nc.vector.tensor_tensor(out=CB4[:, :, :, 12:14],
                                        in0=NIBV, in1=MAb22, op=Alu.mult)

                # ---------- PE: transpose comps (psum -> SBUF via ACT) ------
                for q8 in range(g // 8):           # 8-tile transpose groups
                    ctp = ctp_pool.tile([64, 1024], dt.bfloat16, tag="ctp")
                    for k in range(8):
                        t = 8 * q8 + k
                        nc.tensor.transpose(
                            ctp[:, 128 * k:128 * k + 128],
                            CB[:, t * 64:t * 64 + 64],
                            IDN[:],
                            tile_position=(0, 0))
                    nc.scalar.copy(CT[:, q8 * 1024:(q8 + 1) * 1024], ctp[:])

                # ---------- PE: h matmuls (N=512), relu, layer2 ----------
                RES = rp_pool.tile([128, g * 4], dt.float32, tag="res")
                for q8 in range(g // 8):           # 8-tile groups
                    for pos in range(2):
                        r0 = 32 * pos
                        # 2 psum banks; each bank single-strip (HW constraint)
                        hp = hp_pool.tile([128, 1024], dt.float32, tag="hp")
                        for half in range(2):
                            f0 = q8 * 1024 + half * 512
                            nc.tensor.matmul(
                                hp[:, half * 512:(half + 1) * 512],
                                W16[r0:r0 + 14, :],
                                CT[r0:r0 + 14, f0:f0 + 512],
                                start=True, stop=True,
                                tile_position=(r0, 0))
                        rh0 = q8 * 2048 + pos * 1024
                        nc.scalar.activation(RH[:, rh0:rh0 + 1024], hp[:],
                                             Act.Relu)
                # layer2: res[tok, 2] per (tile, pos)
                for lt in range(g):
                    q8, j = lt // 8, lt % 8
                    for pos in range(2):
                        c0 = q8 * 2048 + pos * 1024 + j * 128
                        nc.tensor.matmul(
                            RES[:, 4 * lt + 2 * pos:4 * lt + 2 * pos + 2],
                            RH[:, c0:c0 + 128],
                            W2[:],
                            start=True, stop=True)

                # ---------- post ----------
                RESS = wpool.tile([128, g * 4], dt.float32, tag="ress")
                nc.vector.tensor_copy(RESS[:], RES[:])
                RESV = RESS[:].rearrange("p (n s w) -> p n s w", s=2, w=2)
                RSEL = wpool.tile([128, g * 2], dt.float32, tag="rsel")
                RSV = RSEL[:].rearrange("p (n s) -> p n s", s=2)
                RD = wpool.tile([128, g * 2], dt.float32, tag="rd")
                RDV = RD[:].rearrange("p (n s) -> p n s", s=2)
                # rsel = res_sub + mA*(res_add - res_sub)
                nc.vector.tensor_tensor(out=RDV, in0=RESV[:, :, :, 0],
                                        in1=RESV[:, :, :, 1], op=Alu.subtract)
                nc.vector.tensor_tensor(
                    out=RDV, in0=RDV,
                    in1=MA[:, :, None].broadcast_to([128, g, 2]), op=Alu.mult)
                nc.vector.tensor_tensor(out=RSV, in0=RDV,
                                        in1=RESV[:, :, :, 1], op=Alu.add)
                # round (RNE), +100, clamp to [100,115]
                nc.vector.tensor_scalar(out=RSEL[:], in0=RSEL[:],
                                        scalar1=ROUND_C,
                                        scalar2=ROUND_C - 100.0,
                                        op0=Alu.add, op1=Alu.subtract)
                nc.vector.tensor_scalar(out=RSEL[:], in0=RSEL[:],
                                        scalar1=100.0, scalar2=115.0,
                                        op0=Alu.max, op1=Alu.min)
                # fold processed mask: r' = r+100-50*m2  (in [0,15] iff m2=2)
                nc.vector.scalar_tensor_tensor(
                    out=RSEL[:],
                    in0=M2[:, :, None].broadcast_to([128, g, 2]),
                    scalar=-50.0, in1=RSV, op0=Alu.mult, op1=Alu.add)
                EQ = wpool.tile([128, g * 32], dt.float32, tag="eq")
                nc.vector.tensor_tensor(
                    out=EQ[:],
                    in0=IOTA[:].rearrange("p (s k) -> p s k", s=2)[:, None]
                        .broadcast_to([128, g, 2, 16]),
                    in1=RSV[:, :, :, None].broadcast_to([128, g, 2, 16]),
                    op=Alu.is_equal)
                nc.vector.scalar_tensor_tensor(
                    out=XR[:, T0:T0 + g, OUT_LO:OUT_LO + 32],
                    in0=EQ[:].rearrange("p (n c) -> p n c", c=32),
                    scalar=2.0,
                    in1=XR[:, T0:T0 + g, OUT_LO:OUT_LO + 32],
                    op0=Alu.mult, op1=Alu.add)

            for d_ in range(ndma):
                t0 = d_ * tiles_per_dma
                nc.sync.dma_start(yr[:, t0:t0 + tiles_per_dma, :],
                                  XR[:, t0:t0 + tiles_per_dma, :])

    nc.compile()
    return nc


def make_consts(W_add1, b_add1, W_add2, b_add2, W_sub1, b_sub1, W_sub2, b_sub2):
    f32 = np.float32
    bf16 = ml_dtypes.bfloat16
    rows = [0, 1, 27, 28]     # GE comps: NIB_A, NIB_B, OP_START+25, OP_START+26

    def eff(W1, b1):
        return np.concatenate([np.asarray(W1, f32)[rows, :],
                               np.asarray(b1, f32)[None, :]], axis=0)

    es = eff(W_sub1, b_sub1)
    ea = eff(W_add1, b_add1)
    blk = np.zeros((10, 128), f32)
    blk[0:5] = es
    blk[5:10] = (ea.astype(np.float64) - es.astype(np.float64)).astype(f32)
    bhi = blk.astype(bf16)
    blo = (blk - bhi.astype(f32)).astype(bf16)
    w16 = np.zeros((128, 128), bf16)
    for s in range(4):
        w16[32 * s:32 * s + 10] = bhi
        w16[32 * s + 10:32 * s + 14] = blo[[0, 1, 5, 6]]

    w2 = np.stack([np.asarray(W_add2, f32)[:, GE_RESULT],
                   np.asarray(W_sub2, f32)[:, GE_RESULT]],
                  axis=1).astype(np.float16)

    iota = np.broadcast_to(np.tile(np.arange(16, dtype=f32), 2), (128, 32)).copy()
    k16 = np.broadcast_to((np.arange(64, dtype=f32) % 16) - 16.0, (128, 64)).copy()
    idn = np.eye(128, dtype=f32).astype(bf16)
    return dict(cW16=w16, cW2=w2, cIOTA=iota, cK16=k16, cID=idn)


_NC_CACHE = {}


def _get_nc(tpc=TPC, g=G):
    key = (tpc, g)
    if key not in _NC_CACHE:
        _NC_CACHE[key] = build_nc(tpc, g)
    return _NC_CACHE[key]


def kernel(x_bd, W_add1, b_add1, W_add2, b_add2, W_sub1, b_sub1, W_sub2, b_sub2):
    from concourse import bass_utils

    x = np.ascontiguousarray(np.asarray(x_bd, dtype=np.float32)).reshape(TOK, D)
    consts = make_consts(W_add1, b_add1, W_add2, b_add2,
                         W_sub1, b_sub1, W_sub2, b_sub2)
    badd2 = float(np.asarray(b_add2)[GE_RESULT])
    bsub2 = float(np.asarray(b_sub2)[GE_RESULT])
    assert badd2 == 0.0 and bsub2 == 0.0, "nonzero output bias not folded"

    nc = _get_nc()
    in_maps = []
    for c in range(NCORES):
        m = dict(consts)
        m["xc"] = x[c * TPC:(c + 1) * TPC]
        in_maps.append(m)
    res = bass_utils.run_bass_kernel_spmd(nc, in_maps, list(range(NCORES)))
    y = np.concatenate([res.results[c]["yc"] for c in range(NCORES)], axis=0)
    return y.reshape(B, S, D)


if __name__ == "__main__":
    build_nc()
    print("built ok")

